# revision 32
# baseline (speedup 1.0000x reference)
"""Trainium2 Bass kernel for nn_DiffusionOrderingNetwork (3-layer GAT, N=50000,
E=800000, softmax over nodes), SPMD across 8 NeuronCores.

Self-contained: host-side index/layout prep + Bass/Tile program + runner.
"""
import sys
sys.path.insert(0, '/opt/trn_rl_repo')
import numpy as np
import ml_dtypes
from contextlib import ExitStack

# ======================= host prep =======================
import numpy as _np

N = 50000
E = 800000
H = 6
C1 = 6
HID = 36
D = 64
NT = 17
NEG = 0.2
NCORES = 8
EPT = 128          # edges per tile
SPT = 8            # node slots per tile
KSUP = 32          # tiles per super-block (layers 1/2) -> 256 psum cols
KSUP3 = 16         # tiles per super-block (layer 3)   -> 128 psum cols


def _fold_ws(W, a):
    # ws[d, h] = sum_c W[d, h*C+c] * a[h, c]
    h, c = a.shape
    return np.einsum('dhc,hc->dh', W.reshape(W.shape[0], h, c), a).astype(np.float32)


def host_prep(x, edge_index, emb, w1, as1, ad1, b1, r1,
              w2, as2, ad2, b2, r2, w3, as3, ad3, b3, r3):
    x = np.asarray(x).astype(np.int64)
    ei = np.asarray(edge_index).astype(np.int64)
    N = x.shape[0]
    NT = emb.shape[0]
    D = emb.shape[1]
    for b in (b1, b2, b3):
        assert np.abs(np.asarray(b)).max() == 0.0, "nonzero bias breaks pad-column math"

    # --- edges with self loops, sorted by dst ---
    src = np.concatenate([ei[0], np.arange(N, dtype=np.int64)])
    dst = np.concatenate([ei[1], np.arange(N, dtype=np.int64)])
    order = np.argsort(dst, kind='stable')
    srcs = src[order]
    dsts = dst[order]
    ET = srcs.shape[0]
    deg = np.bincount(dst, minlength=N).astype(np.int64)
    assert deg.min() >= 1 and deg.max() <= EPT, deg.max()
    node_ptr = np.concatenate([[0], np.cumsum(deg)])  # edge range per node

    # --- shard nodes into NCORES contiguous ranges with ~equal edges ---
    cum = np.cumsum(deg)
    bnds = [0]
    for k in range(1, NCORES):
        bnds.append(int(np.searchsorted(cum, ET * k / NCORES)))
    bnds.append(N)

    # --- per-core greedy tiling: whole nodes, <=EPT edges, <=SPT nodes ---
    core_tiles = []  # per core: list of (first_node, n_nodes)
    for k in range(NCORES):
        nb, ne = bnds[k], bnds[k + 1]
        tiles = []
        cur_first, cur_n, cur_e = nb, 0, 0
        for n in range(nb, ne):
            d = int(deg[n])
            if cur_n == SPT or cur_e + d > EPT:
                tiles.append((cur_first, cur_n))
                cur_first, cur_n, cur_e = n, 0, 0
            cur_n += 1
            cur_e += d
        tiles.append((cur_first, cur_n))
        core_tiles.append(tiles)

    lcm = np.lcm(KSUP, np.lcm(KSUP3, 512 // SPT))  # tiles multiple for chunking
    TMAX = max(len(t) for t in core_tiles)
    TMAX = int(-(-TMAX // lcm) * lcm)
    SLOTMAX = TMAX * SPT
    NMAXOUT = max(bnds[k + 1] - bnds[k] for k in range(NCORES))
    NMAXOUT = int(-(-NMAXOUT // 128) * 128)

    # --- global table-row map ---
    # table rows: [0]=global zero row, then per core k a block of
    # (1+SLOTMAX) rows: block row 0 = core-k zero row, rows 1..SLOTMAX =
    # core-k slots. Core-4's zero row (row 1+4*(SLOTMAX+1)) is tableB's
    # base so int16 gather indices fit both halves.
    S1 = SLOTMAX + 1
    nodeslot = np.zeros(N, dtype=np.int64)
    node_core = np.zeros(N, dtype=np.int64)
    for k in range(NCORES):
        for t, (first, nn) in enumerate(core_tiles[k]):
            ids = np.arange(first, first + nn)
            nodeslot[ids] = 2 + k * S1 + t * SPT + np.arange(nn)
            node_core[ids] = k
    BASE_B = 1 + (NCORES // 2) * S1
    assert BASE_B - 1 <= 32767 and (NCORES // 2) * S1 - 1 <= 32767

    # --- per-core device input arrays ---
    def wrap16(v):
        # dma_gather idx layout: idx i at [i % 16, i // 16], replicated
        # across the 8 gpsimd cores (128 partitions)
        return np.tile(v.reshape(-1, 16).T, (8, 1))

    per_core = []
    for k in range(NCORES):
        tiles = core_tiles[k]
        T = len(tiles)
        nb, ne = bnds[k], bnds[k + 1]
        nreal = ne - nb

        srcgid = np.zeros((TMAX, EPT), dtype=np.int64)   # [tile, edgepos]
        srct17 = np.zeros((TMAX, EPT), dtype=np.int64)
        OH = np.zeros((EPT, TMAX * SPT), dtype=np.float32)
        OHT = np.zeros((SPT, TMAX * EPT), dtype=np.float32)
        OH17T = np.zeros((NT, TMAX * EPT), dtype=np.float32)
        oh17 = np.zeros((NT, SLOTMAX), dtype=np.float32)
        outrowT = np.zeros((128, SLOTMAX // 128), dtype=np.int32) + 10**7

        for t, (first, nn) in enumerate(tiles):
            e0, e1 = node_ptr[first], node_ptr[first + nn]
            ne_t = e1 - e0
            assert ne_t <= EPT
            srcgid[t, :ne_t] = nodeslot[srcs[e0:e1]]
            srct17[t, :ne_t] = 1 + x[srcs[e0:e1]]
            dloc = (dsts[e0:e1] - first).astype(np.int64)
            rows = np.arange(ne_t)
            OH[rows, t * SPT + dloc] = 1.0
            OHT[dloc, t * EPT + rows] = 1.0
            OH17T[x[srcs[e0:e1]], t * EPT + rows] = 1.0
            sl = t * SPT + np.arange(nn)
            oh17[x[first:first + nn], sl] = 1.0
            outrowT[sl % 128, sl // 128] = (np.arange(first, first + nn) - nb)

        # int16 idx streams for the two half-table gathers; dumps hit the
        # zero rows (global row 0 / core-4 zero row) and are summed away.
        ga = np.where(srcgid < BASE_B, srcgid, 0).astype(np.int16)
        gb = np.where(srcgid >= BASE_B, srcgid - BASE_B, 0).astype(np.int16)
        gidx = np.zeros((128, 3, TMAX * SPT), dtype=np.int16)
        gidx[:, 0, :] = wrap16(ga.reshape(TMAX * EPT))
        gidx[:, 1, :] = wrap16(gb.reshape(TMAX * EPT))
        gidx[:, 2, :] = wrap16(srct17.astype(np.int16).reshape(TMAX * EPT))

        npadvec = np.full((D, 1), SLOTMAX - nreal, dtype=np.float32)
        slot_node = np.full(SLOTMAX, -1, dtype=np.int64)
        for t, (first, nn) in enumerate(tiles):
            slot_node[t * SPT:t * SPT + nn] = np.arange(first, first + nn) - nb
        per_core.append(dict(
            gidx=gidx, OH=OH, OHT=OHT, OH17T=OH17T, oh17=oh17,
            outrowT=outrowT, npadvec=npadvec, nreal=nreal, nb=nb, ne=ne,
            slot_node=slot_node,
        ))

    # --- folded weights (shared across cores) ---
    f32 = np.float32
    Wcat1 = np.concatenate([_fold_ws(w1, as1), w1.astype(f32), _fold_ws(w1, ad1)], axis=1)
    Wcat2 = np.concatenate([_fold_ws(w2, as2), w2.astype(f32), _fold_ws(w2, ad2)], axis=1)
    # layer 3: records carry xin itself (identity block); xs3 scores fold w3/as3
    Wcat3 = np.concatenate([_fold_ws(w3, as3), np.eye(HID, dtype=f32), _fold_ws(w3, ad3)], axis=1)
    # W3stack[h*HID+c, o] = w3[c, h*D+o] / H   (mean over heads folded in)
    W3stack = (w3.reshape(HID, H, D).transpose(1, 0, 2).reshape(H * HID, D) / H).astype(f32)
    REP2 = np.zeros((H, HID), dtype=f32)
    REP2[np.arange(HID) // C1, np.arange(HID)] = 1.0
    d3 = np.arange(H * HID)
    REP3A = np.zeros((H, 128), dtype=f32)
    REP3A[d3[:128] // HID, np.arange(128)] = 1.0
    REP3B = np.zeros((H, H * HID - 128), dtype=f32)
    REP3B[d3[128:] // HID, np.arange(H * HID - 128)] = 1.0
    zrow = np.zeros((1, 128), dtype=f32)

    shared = dict(
        emb=emb.astype(f32), embT=emb.astype(f32).T.copy(),
        Wcat1=Wcat1, Wcat2=Wcat2, Wcat3=Wcat3,
        W3stackA=W3stack[:128].copy(), W3stackB=W3stack[128:].copy(),
        r1=r1.astype(f32), r2=r2.astype(f32), r3=r3.astype(f32),
        b1=b1.astype(f32).reshape(-1, 1), b2=b2.astype(f32).reshape(-1, 1),
        b3=b3.astype(f32).reshape(-1, 1),
        REP2=REP2, REP3A=REP3A, REP3B=REP3B, zrow=zrow,
    )
    meta = dict(TMAX=TMAX, SLOTMAX=SLOTMAX, NMAXOUT=NMAXOUT,
                bnds=bnds, nreal=[pc['nreal'] for pc in per_core],
                slot_node=[pc['slot_node'] for pc in per_core])
    return per_core, shared, meta


def numpy_reference(x, edge_index, emb, w1, as1, ad1, b1, r1,
                    w2, as2, ad2, b2, r2, w3, as3, ad3, b3, r3):
    """Plain numpy port of reference.py for quick host validation."""
    def gat(xf, src, dst, W, a_s, a_d, b, r, concat):
        n = xf.shape[0]
        h, c = a_s.shape
        xs = (xf @ W).reshape(n, h, c)
        a_src = (xs * a_s).sum(-1)
        a_dst = (xs * a_d).sum(-1)
        e = a_src[src] + a_dst[dst]
        e = np.where(e > 0, e, NEG * e)
        m = np.full((n, h), -np.inf)
        np.maximum.at(m, dst, e)
        m = np.where(np.isfinite(m), m, 0.0)
        ex = np.exp(e - m[dst])
        s = np.zeros((n, h))
        np.add.at(s, dst, ex)
        alpha = ex / (s[dst] + 1e-16)
        out = np.zeros((n, h, c))
        np.add.at(out, dst, xs[src] * alpha[:, :, None])
        out = out.reshape(n, h * c) if concat else out.mean(1)
        return out + xf @ r + b

    hf = emb[np.asarray(x).astype(np.int64)]
    loops = np.arange(x.shape[0])
    src = np.concatenate([edge_index[0], loops])
    dst = np.concatenate([edge_index[1], loops])
    hf = np.maximum(gat(hf, src, dst, w1, as1, ad1, b1, r1, True), 0)
    hf = np.maximum(gat(hf, src, dst, w2, as2, ad2, b2, r2, True), 0)
    hf = gat(hf, src, dst, w3, as3, ad3, b3, r3, False)
    hf = hf - hf.max(0, keepdims=True)
    e = np.exp(hf)
    return (e / e.sum(0, keepdims=True)).astype(np.float32)


# ======================= device program =======================

import concourse.bass as bass
import concourse.tile as tile
from concourse import bacc, mybir
from concourse.masks import make_identity
from concourse.tile import add_dep_helper

F32 = mybir.dt.float32
I32 = mybir.dt.int32
I16 = mybir.dt.int16
BF16 = mybir.dt.bfloat16

H = 6
EPT = 128
SPT = 8
KSUP = 32      # tiles per super for layers 1/2 (256 psum cols)
KSUP3 = 16     # tiles per super for layer 3  (128 psum cols)


def build_program(TMAX, SLOTMAX, NMAXOUT, D, HID, NT, n_cores=8, edge_dt=BF16,
                  debug_dump=False):
    RW = 48                      # record cols used: asrc(6) | xs(HID=36) | adst(6)
    CW = 128                     # table row width (dma_gather needs 256B elems)
    NCH128 = SLOTMAX // 128
    NCH512 = SLOTMAX // 512
    S1 = SLOTMAX + 1             # per-core table block: zero row + slots
    TROWS = 1 + n_cores * S1
    BASE_B = 1 + max(1, n_cores // 2) * S1   # tableB base (its row 0 = zero row)
    V216 = H * HID               # 216
    VA = 128                     # layer-3 agg split A (dims 0:128)
    VB = V216 - 128              # 88
    cores = list(range(n_cores))

    nc = bacc.Bacc("TRN2", target_bir_lowering=False, debug=False,
                   num_devices=n_cores)

    def din(name, shape, dt=F32):
        return nc.dram_tensor(name, list(shape), dt, kind="ExternalInput")

    gidx_d = din("gidx", [128, 3, TMAX * SPT], I16)
    oh_d = din("OH", [EPT, TMAX * SPT], edge_dt)
    oht_d = din("OHT", [SPT, TMAX * EPT], edge_dt)
    oh17t_d = din("OH17T", [NT, TMAX * EPT], edge_dt)
    oh17_d = din("oh17", [NT, SLOTMAX])
    outr_d = din("outrowT", [128, NCH128], I32)
    npad_d = din("npadvec", [D, 1])
    emb_d = din("emb", [NT, D])
    embt_d = din("embT", [D, NT])
    wcat_d = [din("Wcat1", [D, RW]), din("Wcat2", [HID, RW]), din("Wcat3", [HID, RW])]
    w3a_d = din("W3stackA", [VA, D])
    w3b_d = din("W3stackB", [VB, D])
    r_d = [din("r1", [D, HID]), din("r2", [HID, HID]), din("r3", [HID, D])]
    b_d = [din("b1", [HID, 1]), din("b2", [HID, 1]), din("b3", [D, 1])]
    rep2_d = din("REP2", [H, HID])
    rep3a_d = din("REP3A", [H, VA])
    rep3b_d = din("REP3B", [H, VB])
    zrow_d = din("zrow", [1, CW], edge_dt)
    out_d = nc.dram_tensor("out", [SLOTMAX, D], F32, kind="ExternalOutput")
    if debug_dump:
        dbg1_d = nc.dram_tensor("dbg1", [HID, SLOTMAX], F32, kind="ExternalOutput")
        dbg2_d = nc.dram_tensor("dbg2", [HID, SLOTMAX], F32, kind="ExternalOutput")
        dbg3_d = nc.dram_tensor("dbg3", [D, SLOTMAX], F32, kind="ExternalOutput")

    ag_in = nc.dram_tensor("ag_in", [S1, CW], edge_dt)
    t17d = nc.dram_tensor("t17d", [NT + 1, CW], edge_dt)
    table = nc.dram_tensor("table", [TROWS, CW], edge_dt)
    cca_i = nc.dram_tensor("cca_i", [D, 1], F32)
    cca_o = nc.dram_tensor("cca_o", [D, 1], F32)
    ccs_i = nc.dram_tensor("ccs_i", [D, 1], F32)
    ccs_o = nc.dram_tensor("ccs_o", [D, 1], F32)

    with ExitStack() as ctx:
        tc = ctx.enter_context(tile.TileContext(nc))
        res = ctx.enter_context(tc.tile_pool(name="res", bufs=1))
        cst = ctx.enter_context(tc.tile_pool(name="cst", bufs=1))

        def load(pool, src, shape, dt=F32, tag=None):
            t = pool.tile(list(shape), dt, tag=tag)
            nc.sync.dma_start(out=t[:], in_=src[:])
            return t

        oht_sb = load(res, oh_d, [EPT, TMAX * SPT], edge_dt, tag="oht")
        outr = load(cst, outr_d, [128, NCH128], I32, tag="outr")
        npad_sb = load(cst, npad_d, [D, 1], tag="npad")
        emb_sb = load(cst, emb_d, [NT, D], tag="emb")
        embt_sb = load(cst, embt_d, [D, NT], tag="embt")
        wcat_sb = [load(cst, wcat_d[0], [D, RW], tag="wc1"),
                   load(cst, wcat_d[1], [HID, RW], tag="wc2"),
                   load(cst, wcat_d[2], [HID, RW], tag="wc3")]
        w3a_sb = load(cst, w3a_d, [VA, D], tag="w3a")
        w3b_sb = load(cst, w3b_d, [VB, D], tag="w3b")
        r_sb = [load(cst, r_d[0], [D, HID], tag="r1"),
                load(cst, r_d[1], [HID, HID], tag="r2"),
                load(cst, r_d[2], [HID, D], tag="r3")]
        b_sb = [load(cst, b_d[0], [HID, 1], tag="b1"),
                load(cst, b_d[1], [HID, 1], tag="b2"),
                load(cst, b_d[2], [D, 1], tag="b3")]
        rep2_sb = load(cst, rep2_d, [H, HID], tag="rep2")
        rep3a_sb = load(cst, rep3a_d, [H, VA], tag="rep3a")
        rep3b_sb = load(cst, rep3b_d, [H, VB], tag="rep3b")
        idn = cst.tile([64, 64], F32, tag="idn")
        make_identity(nc, idn[:])
        nc.sync.dma_start(out=table[0:1, :], in_=zrow_d[:])
        agz = nc.sync.dma_start(out=ag_in[0:1, :], in_=zrow_d[:])
        t17z = nc.sync.dma_start(out=t17d[0:1, :], in_=zrow_d[:])

        # ---- h0T = emb.T @ onehot17T ----
        hT0 = res.tile([D, SLOTMAX], F32, tag="h64")
        with tc.tile_pool(name="p0", bufs=2) as p0, \
             tc.tile_pool(name="ps0", bufs=2, space="PSUM") as ps0:
            for c in range(NCH512):
                ohc = p0.tile([NT, 512], F32, tag="ohc")
                nc.sync.dma_start(out=ohc[:], in_=oh17_d[:, c * 512:(c + 1) * 512])
                ps = ps0.tile([D, 512], F32, space="PSUM", tag="ps")
                nc.tensor.matmul(out=ps[:], lhsT=emb_sb[:], rhs=ohc[:],
                                 start=True, stop=True)
                nc.vector.tensor_copy(hT0[:, c * 512:(c + 1) * 512], ps[:])

        # ---- t17 = per-type layer-1 records [NT, RW] ----
        t17_sb = cst.tile([NT, RW], edge_dt, tag="t17")
        with tc.tile_pool(name="p17", bufs=1, space="PSUM") as p17:
            ps = p17.tile([NT, RW], F32, space="PSUM", tag="ps")
            nc.tensor.matmul(out=ps[:], lhsT=embt_sb[:], rhs=wcat_sb[0][:],
                             start=True, stop=True)
            nc.vector.tensor_copy(t17_sb[:], ps[:])
            t17w = nc.sync.dma_start(out=t17d[1:1 + NT, 0:RW], in_=t17_sb[:])

        hT1 = res.tile([HID, SLOTMAX], F32, tag="h36a")
        hT2 = res.tile([HID, SLOTMAX], F32, tag="h36b")
        hins = [hT0, hT1, hT2]
        houts = [hT1, hT2, None]
        agg3A = agg3B = out3T = None
        prev_cc = None
        prev_readers = []

        recbuf = res.tile([128, NCH128 * RW], edge_dt, tag="recbuf")

        for l in range(3):
            hin = hins[l]

            # ---- P1: this core's record-table slice -> recbuf -> ag_in ----
            if l > 0:
                with tc.tile_pool(name=f"recp{l}", bufs=2, space="PSUM") as rpp:
                    for c in range(NCH128):
                        ps = rpp.tile([128, RW], F32, space="PSUM", tag="ps")
                        nc.tensor.matmul(out=ps[:],
                                         lhsT=hin[:, c * 128:(c + 1) * 128],
                                         rhs=wcat_sb[l][:], start=True, stop=True)
                        cp = nc.vector.tensor_copy if c % 2 else nc.scalar.copy
                        cp(recbuf[:, c * RW:(c + 1) * RW], ps[:])
                wdma = nc.sync.dma_start(
                    out=ag_in[1:1 + SLOTMAX, 0:RW].rearrange(
                        "(c p) r -> p c r", p=128),
                    in_=recbuf[:].rearrange("p (c r) -> p c r", r=RW))
                if prev_cc is not None:
                    add_dep_helper(wdma.ins, prev_cc.ins,
                                   reason="ag_in WAR vs previous AllGather")

            # ---- P2: all-gather the record table (layers 2/3 only) ----
            if l > 0:
                if n_cores == 1:
                    cc = nc.sync.dma_start(out=table[1:, :], in_=ag_in[:])
                else:
                    cc = nc.gpsimd.collective_compute(
                        "AllGather", mybir.AluOpType.bypass,
                        replica_groups=[cores],
                        ins=[ag_in[:]], outs=[table[1:, :]],
                    )
                for rd in prev_readers:
                    add_dep_helper(cc.ins, rd.ins,
                                   reason="table WAR vs previous layer gathers")
                prev_cc = cc
                prev_readers = []

            # ---- P3: edge phase ----
            ks = KSUP if l < 2 else KSUP3
            nsup = TMAX // ks
            lw = RW - H if l < 2 else H + V216   # scatter lhsT width: 42 / 222
            cols = ks * SPT                      # psum cols per super
            with tc.tile_pool(name=f"ed{l}", bufs=3 if l == 0 else 2) as wp, \
                 tc.tile_pool(name=f"ix{l}", bufs=2) as ixp, \
                 tc.tile_pool(name=f"edp{l}", bufs=2,
                              space="PSUM") as pp, \
                 tc.tile_pool(name=f"eds{l}", bufs=2, space="PSUM") as pps, \
                 tc.tile_pool(name=f"adp{l}", bufs=2, space="PSUM") as adp:
                CWL = CW
                IXS = 4                      # supers per idx-stream load
                NH = 1 if l == 0 else 2      # idx halves used this layer
                HB = 2 if l == 0 else 0      # gidx half base
                ix4 = None
                for g in range(nsup):
                    t0 = g * ks
                    Rg = wp.tile([EPT, ks * CWL], edge_dt, tag="Rg")
                    R3 = Rg[:].rearrange("p (k e) -> p k e", e=CWL)
                    if True:
                        # per-edge records by dma_gather; layer 0 uses the
                        # 18-row type table (no split), layers 1/2 the two
                        # half tables with zero-row dumps + combine add
                        if g % IXS == 0:
                            ix4 = ixp.tile([128, NH, IXS * ks * SPT], I16,
                                           tag="ix4")
                            nc.sync.dma_start(
                                out=ix4[:],
                                in_=gidx_d[:, HB:HB + NH,
                                           t0 * SPT:(t0 + IXS * ks) * SPT])
                        io = (g % IXS) * ks * SPT
                        RgB = None
                        if l > 0:
                            RgB = wp.tile([EPT, ks * CW], edge_dt, tag="RgB")
                        # SWDGE ring holds 1024 descriptors -> gather in
                        # 8-tile groups (1024 idxs per instruction)
                        GT = 8
                        for q in range(ks // GT):
                            qc = slice(q * GT * CW, (q + 1) * GT * CW)
                            qi = slice(io + q * GT * SPT,
                                       io + (q + 1) * GT * SPT)
                            ga = nc.gpsimd.dma_gather(
                                out_ap=Rg[:, qc].rearrange(
                                    "p (k c) -> p k c", c=CW),
                                in_ap=(t17d[:] if l == 0
                                       else table[0:BASE_B, :]),
                                idxs_ap=ix4[:, 0, qi],
                                num_idxs=GT * EPT, num_idxs_reg=GT * EPT,
                                elem_size=CW)
                            if l == 0:
                                add_dep_helper(ga.ins, t17w.ins,
                                               reason="gather RAW t17 write")
                                add_dep_helper(ga.ins, t17z.ins,
                                               reason="gather RAW t17 zero row")
                                continue
                            gb = nc.gpsimd.dma_gather(
                                out_ap=RgB[:, qc].rearrange(
                                    "p (k c) -> p k c", c=CW),
                                in_ap=table[BASE_B:TROWS, :],
                                idxs_ap=ix4[:, 1, qi],
                                num_idxs=GT * EPT, num_idxs_reg=GT * EPT,
                                elem_size=CW)
                            for gi in (ga, gb):
                                add_dep_helper(gi.ins, prev_cc.ins,
                                               reason="gather RAW AllGather")
                                prev_readers.append(gi)
                        if l > 0:
                            # half-super combines: each waits only on its
                            # own gather groups, overlapping the rest
                            B3 = RgB[:].rearrange("p (k e) -> p k e", e=CW)
                            hk = ks // 2
                            nc.vector.tensor_tensor(
                                out=R3[:, 0:hk, 0:RW], in0=R3[:, 0:hk, 0:RW],
                                in1=B3[:, 0:hk, 0:RW], op=mybir.AluOpType.add)
                            nc.vector.tensor_tensor(
                                out=R3[:, hk:ks, 0:RW], in0=R3[:, hk:ks, 0:RW],
                                in1=B3[:, hk:ks, 0:RW], op=mybir.AluOpType.add)
                    # a_dst computed on-chip: per-tile [SPT, H] matmuls from
                    # resident hin, then one copy to bf16 for the expansion
                    ohts = wp.tile([SPT, ks * EPT], edge_dt, tag="ohts")
                    nc.scalar.dma_start(out=ohts[:],
                                        in_=oht_d[:, t0 * EPT:(t0 + ks) * EPT])
                    AD2 = adp.tile([EPT, 2 * ks * H], F32, space="PSUM",
                                   tag="AD2")
                    psAD = AD2[:, 0:ks * H]
                    psADS = AD2[0:SPT, ks * H:2 * ks * H]
                    for k in range(ks):
                        sl = slice((t0 + k) * SPT, (t0 + k + 1) * SPT)
                        nc.tensor.matmul(out=psADS[:, k * H:(k + 1) * H],
                                         lhsT=hin[:, sl],
                                         rhs=wcat_sb[l][:, RW - H:RW],
                                         start=True, stop=True)
                    ads = wp.tile([SPT, ks * H], edge_dt, tag="ads")
                    nc.scalar.copy(ads[:], psADS[:])
                    for k in range(ks):
                        nc.tensor.matmul(
                            out=AD2[:, k * H:(k + 1) * H],
                            lhsT=ohts[:, k * EPT:(k + 1) * EPT],
                            rhs=ads[:, k * H:(k + 1) * H],
                            start=True, stop=True)
                    esc = wp.tile([EPT, ks * H], F32, tag="esc")
                    nc.vector.tensor_tensor(
                        out=esc[:], in0=R3[:, :, 0:H],
                        in1=psAD, op=mybir.AluOpType.add)
                    nc.vector.scalar_tensor_tensor(
                        out=esc[:], in0=esc[:], scalar=0.2, in1=esc[:],
                        op0=mybir.AluOpType.mult, op1=mybir.AluOpType.max)
                    RHS = wp.tile([EPT, ks * lw], edge_dt, tag="RHS")
                    S3 = RHS[:].rearrange("p (k e) -> p k e", e=lw)
                    nc.scalar.activation(
                        out=S3[:, :, 0:H],
                        in_=esc[:].rearrange("p (k h) -> p k h", h=H),
                        func=mybir.ActivationFunctionType.Exp)
                    ex_rep = S3[:, :, 0:H][:, :, :, None].to_broadcast(
                        [EPT, ks, H, lw // H - 1])
                    if l < 2:
                        xs_in = R3[:, :, H:RW - H].rearrange(
                            "p k (h c) -> p k h c", h=H)
                    else:
                        xs_in = R3[:, :, H:RW - H][:, :, None, :].to_broadcast(
                            [EPT, ks, H, HID])
                    nc.vector.tensor_tensor(
                        out=S3[:, :, H:lw].rearrange("p k (h c) -> p k h c", h=H),
                        in0=xs_in, in1=ex_rep, op=mybir.AluOpType.mult)
                    # S-denominator rows packed with the value scatter psum
                    if l < 2:
                        psSV = pp.tile([64 + HID, cols], F32, space="PSUM",
                                       tag="psSV")
                    else:
                        AB = pp.tile([VA, 2 * cols], F32, space="PSUM",
                                     tag="AB")
                        psA = AB[:, 0:cols]
                        psB = AB[0:VB, cols:2 * cols]
                        S2 = pps.tile([VA, 3 * cols], F32, space="PSUM",
                                      tag="S2")
                    for k in range(ks):
                        t = t0 + k
                        ohs = oht_sb[:, t * SPT:(t + 1) * SPT]
                        lb = k * lw
                        if l < 2:
                            nc.tensor.matmul(
                                out=psSV[0:H, k * SPT:(k + 1) * SPT],
                                lhsT=RHS[:, lb:lb + H], rhs=ohs,
                                start=True, stop=True)
                            nc.tensor.matmul(
                                out=psSV[64:64 + HID, k * SPT:(k + 1) * SPT],
                                lhsT=RHS[:, lb + H:lb + lw], rhs=ohs,
                                start=True, stop=True)
                        else:
                            nc.tensor.matmul(
                                out=S2[0:H, k * SPT:(k + 1) * SPT],
                                lhsT=RHS[:, lb:lb + H], rhs=ohs,
                                start=True, stop=True)
                            nc.tensor.matmul(
                                out=AB[:, k * SPT:(k + 1) * SPT],
                                lhsT=RHS[:, lb + H:lb + H + VA], rhs=ohs,
                                start=True, stop=True)
                            nc.tensor.matmul(
                                out=AB[0:VB, cols + k * SPT:cols + (k + 1) * SPT],
                                lhsT=RHS[:, lb + H + VA:lb + lw], rhs=ohs,
                                start=True, stop=True)
                    psS = psSV[0:H, :] if l < 2 else S2[0:H, 0:cols]
                    rs = wp.tile([H, cols], F32, tag="rs")
                    nc.vector.tensor_scalar_add(out=rs[:], in0=psS,
                                                scalar1=1e-16)
                    nc.vector.reciprocal(out=rs[:], in_=rs[:])
                    csl = slice(g * cols, (g + 1) * cols)
                    if l < 2:
                        ps2 = pp.tile([HID, cols], F32, space="PSUM", tag="ps2")
                        nc.tensor.matmul(out=ps2[:], lhsT=rep2_sb[:], rhs=rs[:],
                                         start=True, stop=True)
                        rr = wp.tile([HID, cols], F32, tag="rr")
                        nc.scalar.copy(out=rr[:], in_=ps2[:])
                        nc.vector.tensor_tensor(
                            out=houts[l][:, csl], in0=psSV[64:64 + HID, :],
                            in1=rr[:], op=mybir.AluOpType.mult)
                    else:
                        nc.tensor.matmul(out=S2[:, cols:2 * cols],
                                         lhsT=rep3a_sb[:],
                                         rhs=rs[:], start=True, stop=True)
                        rrA = wp.tile([VA, cols], F32, tag="rrA")
                        nc.scalar.copy(out=rrA[:], in_=S2[:, cols:2 * cols])
                        nc.vector.tensor_tensor(
                            out=agg3A[:, csl], in0=psA, in1=rrA[:],
                            op=mybir.AluOpType.mult)
                        nc.tensor.matmul(out=S2[0:VB, 2 * cols:3 * cols],
                                         lhsT=rep3b_sb[:],
                                         rhs=rs[:], start=True, stop=True)
                        rrB = wp.tile([VB, cols], F32, tag="rrB")
                        nc.scalar.copy(out=rrB[:], in_=S2[0:VB, 2 * cols:3 * cols])
                        nc.vector.tensor_tensor(
                            out=agg3B[:, csl], in0=psB,
                            in1=rrB[:], op=mybir.AluOpType.mult)

            # ---- P4: residual + bias (+relu) ----
            with tc.tile_pool(name=f"fin{l}", bufs=2, space="PSUM") as fpp:
                for c in range(NCH512):
                    csl = slice(c * 512, (c + 1) * 512)
                    if l < 2:
                        ps = fpp.tile([HID, 512], F32, space="PSUM", tag="ps")
                        nc.tensor.matmul(out=ps[:], lhsT=r_sb[l][:], rhs=hin[:, csl],
                                         start=True, stop=True)
                        nc.vector.tensor_tensor(out=houts[l][:, csl],
                                                in0=houts[l][:, csl], in1=ps[:],
                                                op=mybir.AluOpType.add)
                        nc.scalar.activation(out=houts[l][:, csl],
                                             in_=houts[l][:, csl],
                                             func=mybir.ActivationFunctionType.Relu,
                                             bias=b_sb[l][:])
                    else:
                        ps = fpp.tile([D, 512], F32, space="PSUM", tag="ps64")
                        nc.tensor.matmul(out=ps[:], lhsT=w3a_sb[:],
                                         rhs=agg3A[:, csl], start=True, stop=False)
                        nc.tensor.matmul(out=ps[:], lhsT=w3b_sb[:],
                                         rhs=agg3B[:, csl], start=False, stop=False)
                        nc.tensor.matmul(out=ps[:], lhsT=r_sb[2][:],
                                         rhs=hin[:, csl], start=False, stop=True)
                        nc.vector.tensor_scalar_add(out=out3T[:, csl], in0=ps[:],
                                                    scalar1=b_sb[2][:])

            if debug_dump:
                if l == 0:
                    nc.sync.dma_start(out=dbg1_d[:], in_=hT1[:])
                elif l == 1:
                    nc.sync.dma_start(out=dbg2_d[:], in_=hT2[:])

            if l == 1:
                # layer-3 residents; agg3A/out3T reuse dead slots (h36a/h64)
                agg3A = res.tile([VA, SLOTMAX], F32, tag="h36a")
                agg3B = res.tile([VB, SLOTMAX], F32, tag="agg3B")
                out3T = res.tile([D, SLOTMAX], F32, tag="h64")

        # ---- P5: softmax over nodes (global across cores) ----
        with tc.tile_pool(name="sm", bufs=2) as sp, \
             tc.tile_pool(name="smp", bufs=2, space="PSUM") as spp:
            gmax = sp.tile([D, 1], F32, tag="gmax")
            nc.vector.tensor_reduce(out=gmax[:], in_=out3T[:],
                                    axis=mybir.AxisListType.X,
                                    op=mybir.AluOpType.max)
            nc.sync.dma_start(out=cca_i[:], in_=gmax[:])
            if n_cores == 1:
                cc1 = nc.sync.dma_start(out=cca_o[:], in_=cca_i[:])
            else:
                cc1 = nc.gpsimd.collective_compute(
                    "AllReduce", mybir.AluOpType.max, replica_groups=[cores],
                    ins=[cca_i[:]], outs=[cca_o[:]])
            gmax2 = sp.tile([D, 1], F32, tag="gmax2")
            rb1 = nc.sync.dma_start(out=gmax2[:], in_=cca_o[:])
            add_dep_helper(rb1.ins, cc1.ins, reason="read AllReduce max result")
            negm = sp.tile([D, 1], F32, tag="negm")
            nc.vector.tensor_scalar_mul(out=negm[:], in0=gmax2[:], scalar1=-1.0)
            nc.scalar.activation(out=out3T[:], in_=out3T[:],
                                 func=mybir.ActivationFunctionType.Exp,
                                 bias=negm[:])
            lsum = sp.tile([D, 1], F32, tag="lsum")
            nc.vector.tensor_reduce(out=lsum[:], in_=out3T[:],
                                    axis=mybir.AxisListType.X,
                                    op=mybir.AluOpType.add)
            # pad columns hold exp(b3 - gmax) each; subtract npad of them
            padex = sp.tile([D, 1], F32, tag="padex")
            nc.scalar.activation(out=padex[:], in_=gmax2[:],
                                 func=mybir.ActivationFunctionType.Exp,
                                 bias=b_sb[2][:], scale=-1.0)
            nc.vector.tensor_scalar_mul(out=padex[:], in0=padex[:],
                                        scalar1=npad_sb[:])
            nc.vector.tensor_sub(out=lsum[:], in0=lsum[:], in1=padex[:])
            nc.sync.dma_start(out=ccs_i[:], in_=lsum[:])
            if n_cores == 1:
                cc2 = nc.sync.dma_start(out=ccs_o[:], in_=ccs_i[:])
            else:
                cc2 = nc.gpsimd.collective_compute(
                    "AllReduce", mybir.AluOpType.add, replica_groups=[cores],
                    ins=[ccs_i[:]], outs=[ccs_o[:]])
            gsum = sp.tile([D, 1], F32, tag="gsum")
            rb2 = nc.sync.dma_start(out=gsum[:], in_=ccs_o[:])
            add_dep_helper(rb2.ins, cc2.ins, reason="read AllReduce sum result")
            nc.vector.reciprocal(out=gsum[:], in_=gsum[:])
            nc.vector.tensor_scalar_mul(out=out3T[:], in0=out3T[:],
                                        scalar1=gsum[:])
            if debug_dump:
                nc.sync.dma_start(out=dbg3_d[:], in_=out3T[:])
            for c in range(NCH128):
                trp = spp.tile([128, D], F32, space="PSUM", tag="trp")
                nc.tensor.transpose(out=trp[:],
                                    in_=out3T[:, c * 128:(c + 1) * 128],
                                    identity=idn[:])
                ev = sp.tile([128, D], F32, tag="ev")
                nc.vector.tensor_copy(ev[:], trp[:])
                nc.sync.dma_start(out=out_d[c * 128:(c + 1) * 128, :], in_=ev[:])

    nc.compile()
    return nc


# ======================= runner =======================
_CACHE = {}


def _make_in_maps(per_core, shared):
    ebf = ml_dtypes.bfloat16
    in_maps = []
    for pc in per_core:
        in_maps.append(dict(
            gidx=pc['gidx'],
            OH=pc['OH'].astype(ebf), OHT=pc['OHT'].astype(ebf),
            OH17T=pc['OH17T'].astype(ebf), oh17=pc['oh17'],
            outrowT=pc['outrowT'], npadvec=pc['npadvec'],
            emb=shared['emb'], embT=shared['embT'],
            Wcat1=shared['Wcat1'], Wcat2=shared['Wcat2'],
            Wcat3=shared['Wcat3'], W3stackA=shared['W3stackA'],
            W3stackB=shared['W3stackB'], r1=shared['r1'], r2=shared['r2'],
            r3=shared['r3'], b1=shared['b1'], b2=shared['b2'], b3=shared['b3'],
            REP2=shared['REP2'], REP3A=shared['REP3A'], REP3B=shared['REP3B'],
            zrow=shared['zrow'].astype(ebf),
        ))
    return in_maps


def kernel(x, edge_index, edge_attr=None, **w):
    """Full inputs in, full [50000, 64] float32 softmax output out."""
    from concourse.bass_utils import run_bass_kernel_spmd
    args = dict(x=x, edge_index=edge_index)
    for k in ('emb', 'w1', 'as1', 'ad1', 'b1', 'r1', 'w2', 'as2', 'ad2', 'b2',
              'r2', 'w3', 'as3', 'ad3', 'b3', 'r3'):
        args[k] = np.asarray(w[k])
    per_core, shared, meta = host_prep(**args)
    key = (meta['TMAX'], meta['SLOTMAX'], meta['NMAXOUT'],
           shared['emb'].shape, shared['r2'].shape)
    if key not in _CACHE:
        _CACHE[key] = build_program(
            meta['TMAX'], meta['SLOTMAX'], meta['NMAXOUT'],
            shared['emb'].shape[1], shared['r2'].shape[0],
            shared['emb'].shape[0])
    nc = _CACHE[key]
    in_maps = _make_in_maps(per_core, shared)
    res = run_bass_kernel_spmd(nc, in_maps, list(range(NCORES)))
    D = shared['emb'].shape[1]
    N = meta['bnds'][-1]
    out = np.zeros((N, D), np.float32)
    for k in range(NCORES):
        nb = meta['bnds'][k]
        sn = meta['slot_node'][k]
        real = sn >= 0
        out[nb + sn[real]] = res.results[k]['out'][real]
    return out

